# revision 37
# baseline (speedup 1.0000x reference)
"""Trainium2 Bass kernel for nn_LBLHighwayBiLmV2.

Computation (per batch element b, per layer l in {0,1}):
    fwd = band_fwd(fwd); bwd = band_bwd(bwd)          # 17-tap causal banded mix along L
    fwd = highway(fwd, fW[l], fb[l])  (2 steps)       # x = g*x + (1-g)*relu(nonlin)
    bwd = highway(bwd, bW[l], bb[l])
    if l: fwd += f_cache; bwd += b_cache
    out[l, b] = concat(fwd, bwd, axis=-1)

Sharding: data-parallel over batch, 1 sample per NeuronCore (8 cores).

Per-core dataflow (all matmul operands bf16; PSUM accumulates f32):
  - "home" layout for activations is transposed: vT = [H=512, L=2048] as 4
    SBUF bf16 tiles [128, 2048] (feature chunks on partitions).
  - band: for each 128-wide t-tile, y^T[d,t] = sum_s x_nat[s,d] S[s,t] needs
    only 2 matmuls: the in-tile block (128 cols) plus a 16-col corner from
    the straddling s-tile (the 17-tap band crosses one tile boundary).
    lhsT = x natural (resident bf16 xres for layer 0, crossbar-transposed
    znat for layer 1), rhs = tiny host-built coefficient blocks.
  - highway matmuls: lhsT = W^T chunks (host-pre-transposed bf16), rhs = vT
    -> PSUM [128,1024] tiles grouped (och-chunk, j-pair) so each wide tile
    has one bias: ACT evacuates with fused bias+relu / bias+sigmoid into
    bf16 r/g in a single wide read.
  - gating z = r + g*(v - r) in bf16 split across DVE (2x 16-bit) and Pool.
  - transpose back to natural layout via the DMA crossbar (dma_start_transpose,
    SBUF->SBUF bf16, issued from SP) -- no PE transposes.
  - DRAM output is f32: the bf16 natural tiles are upcast on DVE and stored
    from SP, with each phase's store drain deferred into the next phase so
    it never blocks phase-boundary work.
"""

import os
import sys
from contextlib import ExitStack

import numpy as np

sys.path.insert(0, "/opt/trn_rl_repo")

import concourse.bacc as bacc
import concourse.mybir as mybir
import concourse.tile as tile

F32 = mybir.dt.float32
BF16 = mybir.dt.bfloat16

N_LAYERS = 2
N_HW = 2
B, L, H = 8, 2048, 512
WIDTH = 16
TWO_H = 2 * H
NT = L // 128      # 16 sequence tiles
ND = H // 128      # 4 feature chunks
NJ = L // 512      # 4 t-blocks of 512


# ----------------------------------------------------------------------------
# Host-side parameter prep (tiny, batch-independent)
# ----------------------------------------------------------------------------

def _fwd_band_blocks(w):
    """w: [17] logits (torch slicing convention). Forward score:
    S[s,t] = exp(w[16-(t-s)]) / Z_t for 0 <= t-s <= 16,
    Z_t = sum over valid d of exp(w[16-d]) (d = 0..min(16,t)).

    Returns (bulk[128,128], first[128,128], straddle[128,128]) f32 blocks:
      bulk: in-tile block, full normalization (t >= 17 everywhere).
      first: k=0 block, truncated normalization for t < 17.
      straddle: rows 112..127 hold coeffs from the previous s-tile reaching
      cols 0..15 of the t-tile (padded rows/cols zero).
    """
    w = np.asarray(w, np.float64)
    e = np.exp(w[16 - np.arange(17)] - w.max())      # e[d] = exp(w[16-d]) scaled
    cf = e / e.sum()                                 # full-column weights

    s = np.arange(128)[:, None]
    t = np.arange(128)[None, :]
    d = t - s
    m = (d >= 0) & (d <= 16)
    bulk = np.where(m, cf[np.clip(d, 0, 16)], 0.0)

    Zt = np.cumsum(e)[np.minimum(t, 16)]             # truncated norm for t<17
    first = np.where(m, e[np.clip(d, 0, 16)] / Zt, 0.0)

    # straddle: coeff index = t + 128 - s in [1, 16]
    d2 = t + 128 - s
    straddle = np.where((s >= 112) & (t <= 15) & (d2 >= 1) & (d2 <= 16),
                        cf[np.clip(d2, 0, 16)], 0.0)
    return (bulk.astype(np.float32), first.astype(np.float32),
            straddle.astype(np.float32))


def _bwd_band_blocks(w):
    """Backward: S[s,t] = exp(w[s-t]) / Z_t for 0 <= s-t <= 16,
    Z_t truncated for t > L-17.

    Returns (bulk, last, straddle): straddle rows 0..15 hold coeffs from the
    NEXT s-tile reaching cols 112..127; last is the k=NT-1 block with
    truncated normalization on its high columns.
    """
    w = np.asarray(w, np.float64)
    e = np.exp(w - w.max())                          # e[d] = exp(w[d]) scaled
    cb = e / e.sum()

    s = np.arange(128)[:, None]
    t = np.arange(128)[None, :]
    d = s - t
    m = (d >= 0) & (d <= 16)
    bulk = np.where(m, cb[np.clip(d, 0, 16)], 0.0)

    # last tile: global t = 1920 + t_rel; truncated for t > L-17
    tg = (NT - 1) * 128 + t
    lim = np.minimum(16, (L - 1) - tg)
    Zt = np.cumsum(e)[np.clip(lim, 0, 16)]
    last = np.where(m & (d <= lim), e[np.clip(d, 0, 16)] / Zt, 0.0)

    # straddle: s in next tile (rows 0..15), cols 112..127
    d2 = s + 128 - t
    straddle = np.where((s <= 15) & (t >= 112) & (d2 >= 1) & (d2 <= 16),
                        cb[np.clip(d2, 0, 16)], 0.0)
    return (bulk.astype(np.float32), last.astype(np.float32),
            straddle.astype(np.float32))


def _prep_params(f_scores, b_scores, fW, fb, bW, bb):
    import ml_dtypes
    # bands[l, dir, 128, 3*128]: blocks (bulk | special | straddle), bf16
    bands = np.zeros((N_LAYERS, 2, 128, 384), np.float32)
    for l in range(N_LAYERS):
        fb_, ff_, fs_ = _fwd_band_blocks(f_scores[l])
        bands[l, 0, :, 0:128] = fb_
        bands[l, 0, :, 128:256] = ff_
        bands[l, 0, :, 256:384] = fs_
        bb_, bl_, bs_ = _bwd_band_blocks(b_scores[l])
        bands[l, 1, :, 0:128] = bb_
        bands[l, 1, :, 128:256] = bl_
        bands[l, 1, :, 256:384] = bs_

    # wt[l, dir, h, d, o] = W[l,dir,h][o, d]  (torch Linear weight transposed)
    # bias packed in SBUF layout: [128, (l d h oc)] with
    # bias[p, ((l*2+d)*2+h)*8+oc] = b[l,d,h][oc*128+p]
    wt = np.zeros((N_LAYERS, 2, N_HW, H, TWO_H), np.float32)
    bias = np.zeros((128, N_LAYERS * 2 * N_HW * 8), np.float32)
    for l in range(N_LAYERS):
        for d_, Wsrc, bsrc in ((0, fW, fb), (1, bW, bb)):
            for h in range(N_HW):
                wt[l, d_, h] = np.ascontiguousarray(np.asarray(Wsrc[l, h]).T)
                col0 = ((l * 2 + d_) * N_HW + h) * 8
                bias[:, col0:col0 + 8] = (
                    np.asarray(bsrc[l, h]).reshape(TWO_H // 128, 128).T)
    return (bands.astype(ml_dtypes.bfloat16), wt.astype(ml_dtypes.bfloat16),
            bias)


# ----------------------------------------------------------------------------
# Bass kernel
# ----------------------------------------------------------------------------

def build_nc():
    nc = bacc.Bacc("TRN2", target_bir_lowering=False, debug=False)

    x_ap = nc.dram_tensor("x", [L, H], BF16, kind="ExternalInput").ap()
    wt_ap = nc.dram_tensor("wt", [N_LAYERS, 2, N_HW, H, TWO_H], BF16,
                           kind="ExternalInput").ap()
    bias_ap = nc.dram_tensor("bias", [128, N_LAYERS * 2 * N_HW * 8], F32,
                             kind="ExternalInput").ap()
    bands_ap = nc.dram_tensor("bands", [N_LAYERS, 2, 128, 384], BF16,
                              kind="ExternalInput").ap()
    out_ap = nc.dram_tensor("out", [N_LAYERS, L, TWO_H], F32,
                            kind="ExternalOutput").ap()

    with tile.TileContext(nc) as tc, ExitStack() as ctx:
        const_pool = ctx.enter_context(tc.tile_pool(name="const", bufs=1))
        wt_pool = ctx.enter_context(tc.tile_pool(name="wtp", bufs=8))
        xres_pool = ctx.enter_context(tc.tile_pool(name="xres", bufs=1))
        act_pool = ctx.enter_context(tc.tile_pool(name="act", bufs=1))
        rg_pool = ctx.enter_context(tc.tile_pool(name="rg", bufs=3))
        znat_pool = ctx.enter_context(tc.tile_pool(name="znat", bufs=1))
        zf_pool = ctx.enter_context(tc.tile_pool(name="zf", bufs=4))
        band_psum = ctx.enter_context(tc.tile_pool(name="bdp", bufs=4, space="PSUM"))
        hw_psum = ctx.enter_context(tc.tile_pool(name="hwp", bufs=2, space="PSUM"))

        # load order tuned for the first band phase: its coefficients, then
        # the first x tiles, then everything else
        band_sb = {}
        for l in range(N_LAYERS):
            for d_ in range(2):
                band_sb[(l, d_)] = const_pool.tile(
                    [128, 384], BF16, tag=f"bd{l}{d_}", name=f"bd{l}{d_}")
        bias_sb = const_pool.tile([128, N_LAYERS * 2 * N_HW * 8], F32,
                                  tag="bias")
        xres = [xres_pool.tile([128, H], BF16, tag=f"x{i}", name=f"x{i}")
                for i in range(NT)]

        def xload(i):
            deng = nc.sync if i % 2 == 0 else nc.scalar
            deng.dma_start(xres[i][:], x_ap[i * 128:(i + 1) * 128, :])

        nc.sync.dma_start(band_sb[(0, 0)][:], bands_ap[0, 0])
        for i in range(6):
            xload(i)
        nc.scalar.dma_start(bias_sb[:], bias_ap)
        for i in range(6, NT):
            xload(i)
        nc.sync.dma_start(band_sb[(0, 1)][:], bands_ap[0, 1])
        for l, d_ in ((1, 0), (1, 1)):
            nc.scalar.dma_start(band_sb[(l, d_)][:], bands_ap[l, d_])

        # transposed-layout activation tiles, bf16, separate per direction so
        # fwd/bwd phases have no false dependencies
        def vset(prefix):
            return [act_pool.tile([128, L], BF16, tag=f"{prefix}{c}",
                                  name=f"{prefix}{c}") for c in range(ND)]

        A = {0: vset("Af"), 1: vset("Ab")}
        Bt = {0: vset("Bf"), 1: vset("Bb")}
        C = {0: vset("Cf"), 1: vset("Cb")}
        # natural-layout bf16 [t_p, k, d] (k*512 + d columns) per direction
        znat = {d_: znat_pool.tile([128, NT * H], BF16, tag=f"zn{d_}",
                                   name=f"zn{d_}") for d_ in (0, 1)}

        def band(layer, dir_, src_nat, dst):
            """src_nat: list of NT natural [128, H] bf16 tile APs (s on
            partitions); dst: 4 chunk tiles [128, L] bf16 receiving y^T."""
            bsb = band_sb[(layer, dir_)]
            bulk = bsb[:, 0:128]
            spec = bsb[:, 128:256]      # first (fwd) / last (bwd)
            strd = bsb[:, 256:384]
            for jp in range(NJ):        # j-block of 512 t's = 4 t-tiles
                for dc in range(ND):
                    ps = band_psum.tile([128, 512], F32, tag="bd")
                    for kk in range(4):
                        k = jp * 4 + kk
                        osl = slice(kk * 128, (kk + 1) * 128)
                        if dir_ == 0:
                            main_rhs = spec if k == 0 else bulk
                        else:
                            main_rhs = spec if k == NT - 1 else bulk
                        nc.tensor.matmul(
                            ps[:, osl],
                            src_nat[k][:, dc * 128:(dc + 1) * 128],
                            main_rhs,
                            start=True,
                            stop=not (
                                (dir_ == 0 and k > 0)
                                or (dir_ == 1 and k < NT - 1)),
                        )
                        if dir_ == 0 and k > 0:
                            # 16-col corner from the previous s-tile
                            nc.tensor.matmul(
                                ps[:, kk * 128:kk * 128 + 16],
                                src_nat[k - 1][96:128,
                                               dc * 128:(dc + 1) * 128],
                                strd[96:128, 0:16],
                                start=False, stop=True,
                                tile_position=(96, 0),
                            )
                        elif dir_ == 1 and k < NT - 1:
                            nc.tensor.matmul(
                                ps[:, kk * 128 + 112:(kk + 1) * 128],
                                src_nat[k + 1][0:16,
                                               dc * 128:(dc + 1) * 128],
                                strd[0:16, 112:128],
                                start=False, stop=True,
                                tile_position=(0, 0),
                            )
                    dstap = dst[dc][:, jp * 512:(jp + 1) * 512]
                    if (jp * ND + dc) % 2 == 0:
                        nc.scalar.copy(dstap, ps[:])
                    else:
                        nc.vector.tensor_copy(dstap, ps[:])

        def highway(layer, dir_, h, v, dst, resid=None, mid_hook=None,
                    tail=False):
            """v: input chunk tiles (transposed bf16); dst: output tiles.
            z = r + g*(v - r), r = relu(nl + bn), g = sigmoid(gt + bg).

            PSUM tiles are [128, 1024] spanning a j-pair for one och chunk,
            so ACT evacuates each with a single wide fused bias+act read.
            """
            wts = []
            for dc in range(ND):
                wtile = wt_pool.tile([128, TWO_H], BF16, tag="wt")
                nc.sync.dma_start(
                    wtile[:], wt_ap[layer, dir_, h, dc * 128:(dc + 1) * 128, :])
                wts.append(wtile)
            bcol0 = ((layer * 2 + dir_) * N_HW + h) * 8
            rh = [None] * ND
            gh = [None] * ND
            for jp in range(2):
                if jp == 1 and mid_hook is not None:
                    # the caller transposes dst's first j-pair while the
                    # second half is still computing (shortens the tail)
                    mid_hook()
                for c in range(ND):
                    rh[c] = rg_pool.tile([128, 1024], BF16, tag="rh",
                                         name=f"rh{c}")
                    gh[c] = rg_pool.tile([128, 1024], BF16, tag="gh",
                                         name=f"gh{c}")
                    for part, oc in ((0, c), (1, 4 + c)):
                        ps = hw_psum.tile([128, 1024], F32, tag="rg")
                        for jj in range(2):
                            j = jp * 2 + jj
                            for dc in range(ND):
                                nc.tensor.matmul(
                                    ps[:, jj * 512:(jj + 1) * 512],
                                    wts[dc][:, oc * 128:(oc + 1) * 128],
                                    v[dc][:, j * 512:(j + 1) * 512],
                                    start=(dc == 0),
                                    stop=(dc == ND - 1),
                                )
                        bap = bias_sb[:, bcol0 + oc:bcol0 + oc + 1]
                        tgt = (rh[c] if part == 0 else gh[c])[:]
                        func = (mybir.ActivationFunctionType.Relu if part == 0
                                else mybir.ActivationFunctionType.Sigmoid)
                        nc.scalar.activation(tgt, ps[:], func, bias=bap)
                    # gate this (c, j-pair) slice: z = r + g*(v - r),
                    # split evenly across DVE (2x 16-bit) and Pool.
                    # In the tail (final call, second j-pair) each slice is
                    # halved across BOTH engines to minimize gate latency.
                    if tail and jp == 1:
                        halves = ((nc.vector, slice(jp * 1024, jp * 1024 + 512)),
                                  (nc.gpsimd, slice(jp * 1024 + 512,
                                                    (jp + 1) * 1024)))
                    else:
                        halves = ((nc.vector if c % 2 == 0 else nc.gpsimd,
                                   slice(jp * 1024, (jp + 1) * 1024)),)
                    for eng, sl in halves:
                        tmp = rg_pool.tile([128, 1024], BF16, tag="tmp")
                        tm = tmp[:, 0:(sl.stop - sl.start)]
                        rsl = rh[c][:, sl.start - jp * 1024:sl.stop - jp * 1024]
                        gsl = gh[c][:, sl.start - jp * 1024:sl.stop - jp * 1024]
                        eng.tensor_sub(tm, v[c][:, sl], rsl)
                        eng.tensor_mul(tm, tm, gsl)
                        if resid is None:
                            eng.tensor_add(dst[c][:, sl], tm, rsl)
                        else:
                            eng.tensor_add(tm, tm, rsl)
                            eng.tensor_add(dst[c][:, sl], tm,
                                           resid[c][:, sl])

        def xbar_half(dir_, src, half, quarters=False):
            """Crossbar-transpose one 8-k-tile half of src (4 x [128, L]
            bf16, d on partitions) into znat[dir_] natural [t_p, k, d].
            quarters=True splits each chunk's transpose in two and issues
            from the (by then idle) ACT queue so the first k-tiles land
            earlier without competing with the SP store queue (final
            drain only)."""
            zn3 = znat[dir_][:].rearrange("p (k d) -> p k d", k=NT)
            nq = 2 if quarters else 1
            deng = nc.scalar if quarters else nc.sync
            for q in range(nq):
                cols = 1024 // nq
                k0 = half * 8 + q * (8 // nq)
                ksl = slice(k0, k0 + 8 // nq)
                for c in range(ND):
                    # out[t_p, k, d_in_chunk] = src[c][d, k*128 + t_p]
                    deng.dma_start_transpose(
                        zn3[:, ksl, c * 128:(c + 1) * 128],
                        src[c][:, half * 1024 + q * cols:
                               half * 1024 + (q + 1) * cols])

        def make_emit_outs(layer, dir_, tail=False):
            """Closure emitting the f32 upcast + DRAM stores for a phase.
            The caller defers it into the NEXT phase so the output drain
            never head-of-line-blocks phase-boundary work. In the tail the
            upcasts are split DVE/Pool so the final drain pipelines."""
            zn = znat[dir_]

            def emit_outs():
                for k in range(NT):
                    # ACT must stay free for the PSUM evacuations PE
                    # blocks on, so upcast on DVE (cheap 2x SBUF copy);
                    # in the tail (no compute left) spread the upcasts
                    # over DVE/Pool and the stores over both HWDGE queues
                    zf = zf_pool.tile([128, H], F32, tag="zf")
                    eng = nc.gpsimd if (tail and k % 2) else nc.vector
                    eng.tensor_copy(zf[:], zn[:, k * H:(k + 1) * H])
                    nc.sync.dma_start(
                        out_ap[layer, k * 128:(k + 1) * 128,
                               dir_ * H:(dir_ + 1) * H],
                        zf[:],
                    )

            return emit_outs

        z0 = {}
        pending_outs = None
        for layer in range(N_LAYERS):
            for dir_ in (0, 1):
                src = ([t[:] for t in xres] if layer == 0 else z0[dir_])
                band(layer, dir_, src, A[dir_])
                highway(layer, dir_, 0, A[dir_], Bt[dir_])
                if pending_outs is not None:
                    pending_outs()
                last = layer == 1 and dir_ == 1
                hw_dst = C[dir_] if layer == 0 else A[dir_]
                highway(layer, dir_, 1, Bt[dir_], hw_dst,
                        resid=None if layer == 0 else C[dir_],
                        mid_hook=lambda d=dir_, s=hw_dst: xbar_half(d, s, 0),
                        tail=last)
                xbar_half(dir_, hw_dst, 1)
                pending_outs = make_emit_outs(layer, dir_, tail=last)
                if layer == 0:
                    zn = znat[dir_]
                    z0[dir_] = [zn[:, k * H:(k + 1) * H] for k in range(NT)]
        pending_outs()

    nc.compile()
    return nc


_NC_CACHE = None
LAST_RESULTS = None


def _get_nc():
    global _NC_CACHE
    if _NC_CACHE is None:
        _NC_CACHE = build_nc()
    return _NC_CACHE


def make_in_maps(inputs, f_scores, b_scores, fW, fb, bW, bb):
    import ml_dtypes
    inputs = np.asarray(inputs, np.float32).astype(ml_dtypes.bfloat16)
    bands, wt, bias = _prep_params(
        np.asarray(f_scores), np.asarray(b_scores),
        np.asarray(fW), np.asarray(fb), np.asarray(bW), np.asarray(bb))
    return [
        {"x": np.ascontiguousarray(inputs[b]), "wt": wt, "bias": bias,
         "bands": bands}
        for b in range(B)
    ]


def kernel(inputs, masks, f_scores, b_scores, fW, fb, bW, bb):
    global LAST_RESULTS
    from concourse.bass_utils import run_bass_kernel_spmd

    nc = _get_nc()
    in_maps = make_in_maps(inputs, f_scores, b_scores, fW, fb, bW, bb)
    res = run_bass_kernel_spmd(nc, in_maps, core_ids=list(range(B)),
                               trace=bool(os.environ.get("BASS_TRACE")))
    LAST_RESULTS = res
    out = np.stack([np.asarray(res.results[b]["out"], np.float32)
                    for b in range(B)], axis=1)
    return out
